# revision 12
# baseline (speedup 1.0000x reference)
"""Distributed causal self-attention kernel for 8 TRN2 NeuronCores.

Problem: B=4, T=2048, C=1024, H=16 heads (hs=64), fp32 reference.
  qkv = x @ W_attn + b_attn ; causal softmax attention ; out = y @ W_proj + b_proj

Sharding (Megatron, head-parallel):
  - Each core owns 2 heads (128 of the 1024 C-features).
  - Column-parallel QKV: core c gets W_attn columns for its heads -> [1024, 384].
  - Attention computed fully locally per (batch, head).
  - y^T (local 128 features x 8192 tokens) AllGathered -> Y^T [1024, 8192].
  - Column-parallel proj: core c computes out^T rows [128c:128c+128] (its
    out-column slice) over ALL tokens -> identical SPMD graph, no dynamic
    addressing.  Host concatenates the 8 out^T slices and transposes back.

Layout & precision:
  - Host supplies xT = x^T and the weights in bf16; matmuls run bf16 at
    1 cyc/row with fp32 PSUM accumulation.  Output is fp32.
  - q/k/v all computed feature-major (lhsT=W, rhs=xT); v is then PE-transposed
    to token-major (needed as AV's stationary operand) in [128,128] tiles.
  - Attention computes S^T [keys, queries] = matmul(lhsT=kT, rhs=qT); the
    softmax normalizer comes free by augmenting V with a ones column (row 64
    of the AV PSUM accumulates sum_k P).  exp needs no max-subtraction
    (logits are O(5)).  Causal mask is a bf16 multiplicative mask after exp,
    only on diagonal 4-block groups.
"""

import numpy as np
import ml_dtypes

import concourse.bass as bass
import concourse.mybir as mybir
import concourse.tile as tile
from concourse import bacc
from concourse import bass_utils
from concourse.masks import make_identity

F32 = mybir.dt.float32
BF16 = mybir.dt.bfloat16

B, T, C = 4, 2048, 1024
NH, HS = 16, 64
NCORES = 8
HPC = NH // NCORES          # heads per core = 2
LC = HPC * HS               # local C features per core = 128
NTOK = B * T                # 8192
P = 128
KO = C // P                 # 8 contraction chunks
QC_W = 512                  # query-chunk width (PSUM bank)
KC_W = 128                  # key-chunk width (PSUM partition)
N_QC = T // QC_W            # 4 query chunks per batch
N_KC = T // KC_W            # 16 key chunks per batch
GRP = 4                     # key chunks per exp group ([128, 4, 512] psum)


def build_graph():
    nc = bacc.Bacc(
        "TRN2",
        target_bir_lowering=False,
        debug=False,
        enable_asserts=True,
        num_devices=NCORES,
    )

    xT = nc.dram_tensor("xT", [C, NTOK], BF16, kind="ExternalInput").ap()
    w_qkv = nc.dram_tensor("w_qkv", [C, 3 * LC], BF16, kind="ExternalInput").ap()
    b_qkv = nc.dram_tensor("b_qkv", [3 * LC], F32, kind="ExternalInput").ap()
    w_proj = nc.dram_tensor("w_proj", [C, LC], BF16, kind="ExternalInput").ap()
    b_proj = nc.dram_tensor("b_proj", [LC], F32, kind="ExternalInput").ap()
    out = nc.dram_tensor("out", [LC, NTOK], F32, kind="ExternalOutput").ap()

    xT_t = xT.rearrange("(ko p) t -> p ko t", p=P)          # [128, 8, 8192]
    w_qkv_t = w_qkv.rearrange("(ko p) f -> p ko f", p=P)    # [128, 8, 384]
    w_proj_t = w_proj.rearrange("(ko p) f -> p ko f", p=P)  # [128, 8, 128]

    with tile.TileContext(nc) as tc:
        with (
            tc.tile_pool(name="const", bufs=1) as const,
            tc.tile_pool(name="xslab", bufs=3) as xslab_pool,
            tc.tile_pool(name="qk", bufs=2) as qk_pool,
            tc.tile_pool(name="vtok", bufs=2) as v_pool,
            tc.tile_pool(name="pexp", bufs=3) as p_pool,
            tc.tile_pool(name="small", bufs=4) as small_pool,
            tc.tile_pool(name="outsb", bufs=3) as out_pool,
            tc.tile_pool(name="mm_ps", bufs=2, space="PSUM") as mm_ps,
            tc.tile_pool(name="st_ps", bufs=1, space="PSUM") as st_ps,
            tc.tile_pool(name="y_ps", bufs=1, space="PSUM") as y_ps,
            tc.tile_pool(name="bc_ps", bufs=1, space="PSUM") as bc_ps,
            tc.tile_pool(name="dram", bufs=1, space="DRAM") as dram,
        ):
            # ---- constants ----
            wq_sb = const.tile([P, KO, 3 * LC], BF16)      # QKV weights resident
            nc.sync.dma_start(wq_sb[:], w_qkv_t)
            wp_sb = const.tile([P, KO, LC], BF16)          # proj weights resident
            nc.sync.dma_start(wp_sb[:], w_proj_t)
            bqk_sb = const.tile([P, 3], F32)               # per-partition q/k/v bias
            nc.sync.dma_start(bqk_sb[:], b_qkv.rearrange("(c p) -> p c", p=P))
            bp_sb = const.tile([P, 1], F32)                # proj bias (per-partition)
            nc.sync.dma_start(bp_sb[:], b_proj.rearrange("(c p) -> p c", p=P))
            ones_row = const.tile([1, P], BF16)
            nc.vector.memset(ones_row[:], 1.0)
            ones_col = const.tile([P, N_KC, 1], F32)
            nc.vector.memset(ones_col[:], 1.0)
            ident = const.tile([P, P], BF16)
            make_identity(nc, ident[:])

            # diagonal-group causal mask [k within 4*128 rows, q within 512]
            mask_sb = const.tile([P, GRP, QC_W], BF16)
            nc.gpsimd.memset(mask_sb[:], 1.0)
            for j in range(GRP):
                # keep where q - k - 128*j >= 0 else 0
                nc.gpsimd.affine_select(
                    out=mask_sb[:, j, :],
                    in_=mask_sb[:, j, :],
                    compare_op=mybir.AluOpType.is_ge,
                    fill=0.0,
                    base=-KC_W * j,
                    pattern=[[1, QC_W]],
                    channel_multiplier=-1,
                )

            # DRAM scratch: local y^T bounce and the AllGather result
            y_bounce = dram.tile([LC, NTOK], BF16)
            yT_full = dram.tile([C, NTOK], BF16, addr_space="Shared")

            # ================= per-batch QKV + attention =================
            for b in range(B):
                qT_b = qk_pool.tile([P, N_QC, QC_W], BF16, tag="qT")   # [128,4,512]
                kT_b = qk_pool.tile([P, N_QC, QC_W], BF16, tag="kT")
                vT_b = qk_pool.tile([P, N_QC, QC_W], BF16, tag="vT")
                v_b = v_pool.tile([P, N_KC, 2 * (HS + 1)], BF16, tag="v")
                nc.vector.tensor_copy(v_b[:, :, HS:HS + 1], ones_col[:])
                nc.vector.tensor_copy(v_b[:, :, 2 * HS + 1:], ones_col[:])

                # ---- QKV for this batch, one 512-token slab at a time ----
                for s in range(N_QC):
                    t0 = b * T + s * QC_W
                    slab = xslab_pool.tile([P, KO, QC_W], BF16, tag="xslab")
                    nc.sync.dma_start(slab[:], xT_t[:, :, t0:t0 + QC_W])

                    # q^T / k^T / v^T (feature-major): lhsT = W, rhs = x^T
                    for f, dst in ((0, qT_b), (1, kT_b), (2, vT_b)):
                        ps = mm_ps.tile([P, QC_W], F32, tag="mm")
                        for k0 in range(KO):
                            nc.tensor.matmul(
                                ps[:],
                                wq_sb[:, k0, f * P:(f + 1) * P],
                                slab[:, k0, :],
                                start=(k0 == 0),
                                stop=(k0 == KO - 1),
                            )
                        nc.vector.tensor_tensor(
                            dst[:, s, :], ps[:],
                            bqk_sb[:, f:f + 1].to_broadcast((P, QC_W)),
                            mybir.AluOpType.add,
                        )

                    # transpose v^T -> token-major v_b [128 tok, {64v|1|64v|1}]
                    for t4 in range(QC_W // P):
                        tps = mm_ps.tile([P, P], BF16, tag="mm")
                        nc.tensor.transpose(
                            tps[:], vT_b[:, s, t4 * P:(t4 + 1) * P], ident[:],
                        )
                        kc = s * (QC_W // P) + t4
                        nc.vector.tensor_copy(v_b[:, kc, 0:HS], tps[:, 0:HS])
                        nc.vector.tensor_copy(
                            v_b[:, kc, HS + 1:2 * HS + 1], tps[:, HS:2 * HS])

                # ---- attention for this batch, per local head ----
                for h in range(HPC):
                    hp = h * HS          # partition offset of head in qT/kT
                    vc = h * (HS + 1)    # column offset of head in v_b
                    for qc in range(N_QC):
                        yps = y_ps.tile([P, QC_W], F32, tag="y")
                        ngrp = qc + 1
                        for g in range(ngrp):
                            stps = st_ps.tile([P, GRP, QC_W], F32, tag="st")
                            for j in range(GRP):
                                kc = g * GRP + j
                                s_idx, sub = kc // GRP, kc % GRP
                                nc.tensor.matmul(
                                    stps[:, j, :],
                                    kT_b[hp:hp + HS, s_idx,
                                         sub * KC_W:(sub + 1) * KC_W],
                                    qT_b[hp:hp + HS, qc, :],
                                    start=True, stop=True,
                                )
                            pexp = p_pool.tile([P, GRP, QC_W], BF16, tag="p")
                            nc.scalar.activation(
                                pexp[:], stps[:],
                                mybir.ActivationFunctionType.Exp,
                                scale=1.0 / np.sqrt(HS),
                            )
                            if g == qc:  # diagonal group: zero out k > q
                                nc.vector.tensor_tensor(
                                    pexp[:], pexp[:], mask_sb[:],
                                    mybir.AluOpType.mult,
                                )
                            for j in range(GRP):
                                kc = g * GRP + j
                                nc.tensor.matmul(
                                    yps[0:HS + 1, :],
                                    v_b[:, kc, vc:vc + HS + 1],
                                    pexp[:, j, :],
                                    start=(g == 0 and j == 0),
                                    stop=(g == ngrp - 1 and j == GRP - 1),
                                )
                        # normalize: recip of sums row, broadcast via K=1 matmul
                        recip = small_pool.tile([1, QC_W], BF16, tag="recip")
                        with nc.allow_low_precision(
                                reason="bf16 softmax normalizer is within tolerance"):
                            nc.vector.reciprocal(recip[:], yps[HS:HS + 1, :])
                        bcps = bc_ps.tile([HS, QC_W], F32, tag="bc")
                        nc.tensor.matmul(bcps[:], ones_row[:, :HS], recip[:],
                                         start=True, stop=True)
                        bc_sb = small_pool.tile([HS, QC_W], F32, tag="bc_sb")
                        nc.vector.tensor_copy(bc_sb[:], bcps[:])
                        yout = out_pool.tile([HS, QC_W], BF16, tag="yout")
                        nc.vector.tensor_tensor(
                            yout[:], yps[0:HS, :], bc_sb[:], mybir.AluOpType.mult,
                        )
                        nc.sync.dma_start(
                            y_bounce[hp:hp + HS, b * T + qc * QC_W:
                                     b * T + (qc + 1) * QC_W],
                            yout[:],
                        )

            # ================= AllGather y^T =================
            nc.gpsimd.collective_compute(
                "AllGather",
                mybir.AluOpType.bypass,
                ins=[y_bounce.opt()],
                outs=[yT_full.opt()],
                replica_groups=[list(range(NCORES))],
            )

            # ================= output projection (column slice) =================
            yT_t = yT_full[:].rearrange("(ko p) t -> p ko t", p=P)
            for tn in range(NTOK // QC_W):
                yslab = xslab_pool.tile([P, KO, QC_W], BF16, tag="yslab")
                nc.sync.dma_start(yslab[:], yT_t[:, :, tn * QC_W:(tn + 1) * QC_W])
                ps = mm_ps.tile([P, QC_W], F32, tag="mm")
                for k0 in range(KO):
                    nc.tensor.matmul(
                        ps[:],
                        wp_sb[:, k0, :],
                        yslab[:, k0, :],
                        start=(k0 == 0),
                        stop=(k0 == KO - 1),
                    )
                osb = out_pool.tile([P, QC_W], F32, tag="osb")
                nc.vector.tensor_tensor(
                    osb[:], ps[:], bp_sb[:, 0:1].to_broadcast((P, QC_W)),
                    mybir.AluOpType.add,
                )
                nc.sync.dma_start(out[:, tn * QC_W:(tn + 1) * QC_W], osb[:])

    nc.compile()
    return nc


_NC_CACHE = None


def _get_nc():
    global _NC_CACHE
    if _NC_CACHE is None:
        _NC_CACHE = build_graph()
    return _NC_CACHE


def make_in_maps(x, W_attn, b_attn, W_proj, b_proj):
    x = np.asarray(x, dtype=np.float32)
    W_attn = np.asarray(W_attn, dtype=np.float32)
    b_attn = np.asarray(b_attn, dtype=np.float32)
    W_proj = np.asarray(W_proj, dtype=np.float32)
    b_proj = np.asarray(b_proj, dtype=np.float32)

    bf = ml_dtypes.bfloat16
    xT = np.ascontiguousarray(x.reshape(NTOK, C).T).astype(bf)  # [1024, 8192]
    in_maps = []
    for c in range(NCORES):
        sl = slice(LC * c, LC * (c + 1))
        w_loc = np.ascontiguousarray(np.concatenate(
            [W_attn[:, 0 * C:][:, sl], W_attn[:, 1 * C:][:, sl],
             W_attn[:, 2 * C:][:, sl]], axis=1)).astype(bf)  # [1024, 384]
        b_loc = np.ascontiguousarray(np.concatenate(
            [b_attn[0 * C:][sl], b_attn[1 * C:][sl], b_attn[2 * C:][sl]]))
        wp_loc = np.ascontiguousarray(W_proj[:, sl]).astype(bf)  # [1024, 128]
        bp_loc = np.ascontiguousarray(b_proj[sl])
        in_maps.append({
            "xT": xT, "w_qkv": w_loc, "b_qkv": b_loc,
            "w_proj": wp_loc, "b_proj": bp_loc,
        })
    return in_maps


def kernel(x, W_attn, b_attn, W_proj, b_proj):
    nc = _get_nc()
    in_maps = make_in_maps(x, W_attn, b_attn, W_proj, b_proj)
    res = bass_utils.run_bass_kernel_spmd(
        nc, in_maps, core_ids=list(range(NCORES)), trace=False,
    )
    outT = np.concatenate([res.results[c]["out"] for c in range(NCORES)], axis=0)
    out = np.ascontiguousarray(outT.T).reshape(B, T, C).astype(np.float32)
    kernel.last_results = res
    return out


# revision 16
# speedup vs baseline: 3555.2525x; 3555.2525x over previous
"""Collective-free sequence-sharded causal self-attention for 8 TRN2 cores.

Sharding: core c -> batch b = c//2, zig-zag half z = c%2.  The core computes
ALL 16 heads for two 512-token query blocks of its batch:
    block A: queries [512z, 512z+512)        (kv extent 1024, mask_a)
    block B: queries [2048-512(z+1), ...+512) (kv extent 2048; kv<1024 is
             fully causal -> unmasked; kv in [1024,2048) uses mask_b)
The zig-zag pairing makes every core's instruction graph IDENTICAL (SPMD)
while balancing true causal work; causality differences live in per-core
mask DATA supplied by the host.  K/V for the batch's first 2048 tokens are
recomputed on both cores of a pair (cheaper than any collective here).

Everything runs bf16 with fp32 PSUM accumulation; softmax normalizer via a
ones column appended per head to V (row 64 of the AV PSUM = sum_k P); no
max-subtraction (logits are O(5)).  y^T stays in SBUF; the projection for
the core's own tokens reads it directly.  Output is the core's 1024 token
rows of the final [8192, 1024], reassembled by the host.
"""

import numpy as np
import ml_dtypes

import concourse.bass as bass
import concourse.mybir as mybir
import concourse.tile as tile
from concourse import bacc
from concourse import bass_utils
from concourse.masks import make_identity

F32 = mybir.dt.float32
BF16 = mybir.dt.bfloat16

B, T, C = 4, 2048, 1024
NH, HS = 16, 64
NCORES = 8
NTOK = B * T
P = 128
KO = C // P                 # 8 contraction chunks over C
QW = 512                    # query block width
KC_W = 128                  # kv chunk width (PSUM partition)
GRP = 2                     # kv chunks per exp group
KV_A, KV_B = 1024, 2048     # kv extents of block A / block B
NFC = NH // 2               # 8 feature chunks of 128 (2 heads each)


def build_graph():
    nc = bacc.Bacc(
        "TRN2",
        target_bir_lowering=False,
        debug=False,
        enable_asserts=True,
        num_devices=NCORES,
    )

    xq = nc.dram_tensor("xq", [C, 2 * QW], BF16, kind="ExternalInput").ap()
    xkv = nc.dram_tensor("xkv", [C, KV_B], BF16, kind="ExternalInput").ap()
    w_qkv = nc.dram_tensor("w_qkv", [C, 3 * C], BF16, kind="ExternalInput").ap()
    b_qkv = nc.dram_tensor("b_qkv", [3 * C], F32, kind="ExternalInput").ap()
    w_proj = nc.dram_tensor("w_proj", [C, C], BF16, kind="ExternalInput").ap()
    b_proj = nc.dram_tensor("b_proj", [C], F32, kind="ExternalInput").ap()
    mask_a = nc.dram_tensor("mask_a", [KV_A, QW], BF16, kind="ExternalInput").ap()
    mask_b = nc.dram_tensor("mask_b", [KV_A, QW], BF16, kind="ExternalInput").ap()
    out = nc.dram_tensor("out", [2 * QW, C], F32, kind="ExternalOutput").ap()

    xq_t = xq.rearrange("(ko p) t -> p ko t", p=P)        # [128, 8, 1024]
    xkv_t = xkv.rearrange("(ko p) t -> p ko t", p=P)      # [128, 8, 2048]
    wq_t = w_qkv.rearrange("(ko p) f -> p ko f", p=P)     # [128, 8, 3072]
    wp_t = w_proj.rearrange("(ko p) f -> p ko f", p=P)    # [128, 8, 1024]
    ma_t = mask_a.rearrange("(kc p) q -> p kc q", p=P)    # [128, 8, 512]
    mb_t = mask_b.rearrange("(kc p) q -> p kc q", p=P)    # [128, 8, 512]

    with tile.TileContext(nc) as tc:
        with (
            tc.tile_pool(name="const", bufs=1) as const,
            tc.tile_pool(name="w", bufs=1) as w_pool,
            tc.tile_pool(name="xslab", bufs=2) as xslab_pool,
            tc.tile_pool(name="qk", bufs=1) as qk_pool,
            tc.tile_pool(name="vt", bufs=1) as vt_pool,
            tc.tile_pool(name="vtok", bufs=1) as v_pool,
            tc.tile_pool(name="pexp", bufs=4) as p_pool,
            tc.tile_pool(name="small", bufs=2) as small_pool,
            tc.tile_pool(name="outsb", bufs=2) as out_pool,
            tc.tile_pool(name="mm_ps", bufs=2, space="PSUM") as mm_ps,
            tc.tile_pool(name="st_ps", bufs=2, space="PSUM") as st_ps,
            tc.tile_pool(name="y_ps", bufs=2, space="PSUM") as y_ps,
        ):
            # ---- small constants ----
            bqk_sb = const.tile([P, 24], F32)      # qkv bias, per-partition
            nc.sync.dma_start(bqk_sb[:], b_qkv.rearrange("(c p) -> p c", p=P))
            bp_row = const.tile([1, C], F32)
            nc.sync.dma_start(bp_row[:], b_proj[None, :])
            ones_row = const.tile([1, P], BF16)
            nc.vector.memset(ones_row[:], 1.0)
            ones_col = const.tile([P, KV_B // P, 1], F32)
            nc.vector.memset(ones_col[:], 1.0)
            ident = const.tile([P, P], BF16)
            make_identity(nc, ident[:])

            # proj bias broadcast across partitions: [1,1024] -> [128,1024]
            bp_row16 = const.tile([1, C], BF16)
            nc.vector.tensor_copy(bp_row16[:], bp_row[:])
            bp_bc = const.tile([P, C], F32)
            for half in range(2):
                bp_ps = mm_ps.tile([P, QW], F32, tag="mm")
                nc.tensor.matmul(bp_ps[:], ones_row[:],
                                 bp_row16[:, half * QW:(half + 1) * QW],
                                 start=True, stop=True)
                nc.vector.tensor_copy(bp_bc[:, half * QW:(half + 1) * QW], bp_ps[:])

            # ---- QKV pass 1a: q^T for block A only ----
            qT = qk_pool.tile([P, NFC, 2 * QW], BF16, tag="qT")
            w_q = w_pool.tile([P, KO, C], BF16, tag="w", name="w_q")
            for fq in range(NFC):
                nc.sync.dma_start(w_q[:, :, fq * P:(fq + 1) * P],
                                  wq_t[:, :, fq * P:(fq + 1) * P])

            def qt_slab(s, w_tile):
                slab = xslab_pool.tile([P, KO, QW], BF16, tag="xslab",
                                       name=f"xq{s}")
                nc.sync.dma_start(slab[:], xq_t[:, :, s * QW:(s + 1) * QW])
                for f in range(NFC):
                    ps = mm_ps.tile([P, QW], F32, tag="mm")
                    for k0 in range(KO):
                        nc.tensor.matmul(
                            ps[:], w_tile[:, k0, f * P:(f + 1) * P],
                            slab[:, k0, :],
                            start=(k0 == 0), stop=(k0 == KO - 1),
                        )
                    nc.scalar.activation(
                        qT[:, f, s * QW:(s + 1) * QW], ps[:],
                        mybir.ActivationFunctionType.Identity,
                        bias=bqk_sb[:, f:f + 1],
                    )

            qt_slab(0, w_q)

            # ---- QKV pass 2: k^T and v over 2048 kv tokens ----
            kT = qk_pool.tile([P, NFC, KV_B], BF16, tag="kT")
            v_aug = v_pool.tile([P, KV_B // P, NH * (HS + 1)], BF16, tag="v")
            for h in range(NH):
                nc.vector.tensor_copy(
                    v_aug[:, :, h * (HS + 1) + HS:h * (HS + 1) + HS + 1],
                    ones_col[:])
            w_kv = w_pool.tile([P, KO, 2 * C], BF16, tag="w", name="w_kv")
            for fq in range(2 * NFC):
                nc.sync.dma_start(w_kv[:, :, fq * P:(fq + 1) * P],
                                  wq_t[:, :, C + fq * P:C + (fq + 1) * P])

            def kv_slab(s):
                slab = xslab_pool.tile([P, KO, QW], BF16, tag="xslab",
                                       name=f"xkv{s}")
                nc.sync.dma_start(slab[:], xkv_t[:, :, s * QW:(s + 1) * QW])
                for f in range(NFC):
                    ps = mm_ps.tile([P, QW], F32, tag="mm")
                    for k0 in range(KO):
                        nc.tensor.matmul(
                            ps[:], w_kv[:, k0, f * P:(f + 1) * P],
                            slab[:, k0, :],
                            start=(k0 == 0), stop=(k0 == KO - 1),
                        )
                    nc.scalar.activation(
                        kT[:, f, s * QW:(s + 1) * QW], ps[:],
                        mybir.ActivationFunctionType.Identity,
                        bias=bqk_sb[:, 8 + f:9 + f],
                    )
                vt_slab = vt_pool.tile([P, NFC, QW], BF16, tag="vt")
                for f in range(NFC):
                    ps = mm_ps.tile([P, QW], F32, tag="mm")
                    for k0 in range(KO):
                        nc.tensor.matmul(
                            ps[:], w_kv[:, k0, C + f * P:C + (f + 1) * P],
                            slab[:, k0, :],
                            start=(k0 == 0), stop=(k0 == KO - 1),
                        )
                    nc.scalar.activation(
                        vt_slab[:, f, :], ps[:],
                        mybir.ActivationFunctionType.Identity,
                        bias=bqk_sb[:, 16 + f:17 + f],
                    )
                for f in range(NFC):
                    for t4 in range(QW // P):
                        tps = mm_ps.tile([P, P], BF16, tag="mm")
                        nc.tensor.transpose(
                            tps[:], vt_slab[:, f, t4 * P:(t4 + 1) * P], ident[:])
                        tc_i = s * (QW // P) + t4
                        h0, h1 = 2 * f, 2 * f + 1
                        nc.vector.tensor_copy(
                            v_aug[:, tc_i, h0 * (HS + 1):h0 * (HS + 1) + HS],
                            tps[:, 0:HS])
                        nc.vector.tensor_copy(
                            v_aug[:, tc_i, h1 * (HS + 1):h1 * (HS + 1) + HS],
                            tps[:, HS:2 * HS])

            kv_slab(0)
            kv_slab(1)
            # masks can load behind the first kv slabs
            ma_sb = const.tile([P, KV_A // P, QW], BF16)
            nc.sync.dma_start(ma_sb[:], ma_t)
            mb_sb = const.tile([P, KV_A // P, QW], BF16)
            nc.sync.dma_start(mb_sb[:], mb_t)

            # ---- attention: head pairs packed per feature chunk ----
            # For each fc f, heads e=2f (partitions 0:64) and o=2f+1 (64:128)
            # issue S^T matmuls into slabs j=0/1 of one [128,2,512] PSUM tile
            # with tile_position (0,0)/(64,0) -> concurrent PE row-groups.
            # One exp + one (broadcast) mask op covers both heads.
            yT = qk_pool.tile([P, NFC, 2 * QW], BF16, tag="yT")  # y^T local

            def attn_block(f, blk, nkc, masked_from):
                yps_e = y_ps.tile([P, QW], F32, tag="y", name=f"y_e_{f}_{blk}")
                yps_o = y_ps.tile([P, QW], F32, tag="y", name=f"y_o_{f}_{blk}")
                for kc in range(nkc):
                    stps = st_ps.tile([P, 2, QW], F32, tag="st")
                    for j, hp in ((0, 0), (1, HS)):
                        nc.tensor.matmul(
                            stps[:, j, :],
                            kT[hp:hp + HS, f, kc * KC_W:(kc + 1) * KC_W],
                            qT[hp:hp + HS, f, blk * QW:(blk + 1) * QW],
                            start=True, stop=True,
                            tile_position=(hp, 0),
                        )
                    pexp = p_pool.tile([P, 2, QW], BF16, tag="p")
                    nc.scalar.activation(
                        pexp[:], stps[:],
                        mybir.ActivationFunctionType.Exp,
                        scale=1.0 / np.sqrt(HS),
                    )
                    if blk == 0 or kc >= masked_from:
                        m2 = ma_sb if blk == 0 else mb_sb
                        kcm = kc if blk == 0 else kc - masked_from
                        nc.vector.tensor_tensor(
                            pexp[:], pexp[:],
                            m2[:, kcm:kcm + 1, :].to_broadcast((P, 2, QW)),
                            mybir.AluOpType.mult)
                    for j, h in ((0, 2 * f), (1, 2 * f + 1)):
                        vc = h * (HS + 1)
                        yps = yps_e if j == 0 else yps_o
                        nc.tensor.matmul(
                            yps[0:HS + 1, :],
                            v_aug[:, kc, vc:vc + HS + 1],
                            pexp[:, j, :],
                            start=(kc == 0), stop=(kc == nkc - 1),
                        )
                for j, hp in ((0, 0), (1, HS)):
                    yps = yps_e if j == 0 else yps_o
                    # evict to SBUF right away so the PSUM bank frees for the
                    # next head pair; normalize from SBUF afterwards
                    y_sb = small_pool.tile([HS + 1, QW], F32, tag="y_sb")
                    nc.vector.tensor_copy(y_sb[:], yps[0:HS + 1, :])
                    recip = small_pool.tile([1, QW], BF16, tag="recip")
                    with nc.allow_low_precision(
                            reason="bf16 softmax normalizer within tolerance"):
                        nc.vector.reciprocal(recip[:], y_sb[HS:HS + 1, :])
                    bcps = mm_ps.tile([HS, QW], F32, tag="mm")
                    nc.tensor.matmul(bcps[:], ones_row[:, :HS], recip[:],
                                     start=True, stop=True)
                    bc_sb = small_pool.tile([HS, QW], F32, tag="bc_sb")
                    nc.vector.tensor_copy(bc_sb[:], bcps[:])
                    nc.vector.tensor_tensor(
                        yT[hp:hp + HS, f, blk * QW:(blk + 1) * QW],
                        y_sb[0:HS, :], bc_sb[:], mybir.AluOpType.mult,
                    )

            # block A only needs kv slabs 0-1: interleave its head pairs with
            # the remaining kv production and the block-B q^T pass
            for f in range(NFC // 2):
                attn_block(f, 0, KV_A // KC_W, KV_A // KC_W)
            kv_slab(2)
            for f in range(NFC // 2, NFC):
                attn_block(f, 0, KV_A // KC_W, KV_A // KC_W)
            kv_slab(3)
            w_q2 = w_pool.tile([P, KO, C], BF16, tag="w", name="w_q2")
            for fq in range(NFC):
                nc.sync.dma_start(w_q2[:, :, fq * P:(fq + 1) * P],
                                  wq_t[:, :, fq * P:(fq + 1) * P])
            qt_slab(1, w_q2)
            for f in range(NFC):
                attn_block(f, 1, KV_B // KC_W, KV_A // KC_W)

            # ---- projection for own 1024 tokens (all output columns) ----
            w_p = w_pool.tile([P, KO, C], BF16, tag="w", name="w_p")
            nc.sync.dma_start(w_p[:], wp_t)
            for tm in range(2 * QW // P):      # 8 token chunks of 128
                for nn in range(C // QW):      # 2 column chunks of 512
                    ps = mm_ps.tile([P, QW], F32, tag="mm")
                    for k0 in range(KO):
                        nc.tensor.matmul(
                            ps[:],
                            yT[:, k0, tm * P:(tm + 1) * P],
                            w_p[:, k0, nn * QW:(nn + 1) * QW],
                            start=(k0 == 0), stop=(k0 == KO - 1),
                        )
                    osb = out_pool.tile([P, QW], F32, tag="osb")
                    nc.vector.tensor_tensor(
                        osb[:], ps[:], bp_bc[:, nn * QW:(nn + 1) * QW],
                        mybir.AluOpType.add,
                    )
                    nc.sync.dma_start(
                        out[tm * P:(tm + 1) * P, nn * QW:(nn + 1) * QW], osb[:])

    nc.compile()
    return nc


_NC_CACHE = None


def _get_nc():
    global _NC_CACHE
    if _NC_CACHE is None:
        _NC_CACHE = build_graph()
    return _NC_CACHE


def _q_ranges(c):
    """Global token rows (within [0, 8192)) of core c's blocks A and B."""
    b, z = c // 2, c % 2
    a0 = b * T + 512 * z
    b0 = b * T + T - 512 * (z + 1)
    return (a0, a0 + QW), (b0, b0 + QW)


def make_in_maps(x, W_attn, b_attn, W_proj, b_proj):
    x = np.asarray(x, dtype=np.float32)
    W_attn = np.asarray(W_attn, dtype=np.float32)
    b_attn = np.asarray(b_attn, dtype=np.float32)
    W_proj = np.asarray(W_proj, dtype=np.float32)
    b_proj = np.asarray(b_proj, dtype=np.float32)

    bf = ml_dtypes.bfloat16
    xT = np.ascontiguousarray(x.reshape(NTOK, C).T).astype(bf)  # [1024, 8192]
    wq = np.ascontiguousarray(W_attn).astype(bf)
    wp = np.ascontiguousarray(W_proj).astype(bf)
    kv = np.arange(KV_B)

    in_maps = []
    for c in range(NCORES):
        b, z = c // 2, c % 2
        (a0, a1), (b0, b1) = _q_ranges(c)
        xq_c = np.ascontiguousarray(
            np.concatenate([xT[:, a0:a1], xT[:, b0:b1]], axis=1))
        xkv_c = np.ascontiguousarray(xT[:, b * T:b * T + KV_B])
        qa = np.arange(a0 - b * T, a1 - b * T)   # q positions within batch
        qb = np.arange(b0 - b * T, b1 - b * T)
        m_a = (kv[:KV_A, None] <= qa[None, :]).astype(bf)           # [1024,512]
        m_b = (kv[KV_A:, None] <= qb[None, :]).astype(bf)           # [1024,512]
        in_maps.append({
            "xq": xq_c, "xkv": xkv_c,
            "w_qkv": wq, "b_qkv": b_attn,
            "w_proj": wp, "b_proj": b_proj,
            "mask_a": np.ascontiguousarray(m_a),
            "mask_b": np.ascontiguousarray(m_b),
        })
    return in_maps


def kernel(x, W_attn, b_attn, W_proj, b_proj):
    nc = _get_nc()
    in_maps = make_in_maps(x, W_attn, b_attn, W_proj, b_proj)
    res = bass_utils.run_bass_kernel_spmd(
        nc, in_maps, core_ids=list(range(NCORES)), trace=False,
    )
    out_full = np.empty((NTOK, C), dtype=np.float32)
    for c in range(NCORES):
        (a0, a1), (b0, b1) = _q_ranges(c)
        blk = res.results[c]["out"]
        out_full[a0:a1] = blk[:QW]
        out_full[b0:b1] = blk[QW:]
    kernel.last_results = res
    return out_full.reshape(B, T, C)


# revision 17
# speedup vs baseline: 4469.5139x; 1.2572x over previous
"""Collective-free sequence-sharded causal self-attention for 8 TRN2 cores.

Sharding: core c -> batch b = c//2, zig-zag half z = c%2.  The core computes
ALL 16 heads for two 512-token query blocks of its batch:
    block A: queries [512z, 512z+512)        (kv extent 1024, mask_a)
    block B: queries [2048-512(z+1), ...+512) (kv extent 2048; kv<1024 is
             fully causal -> unmasked; kv in [1024,2048) uses mask_b)
The zig-zag pairing makes every core's instruction graph IDENTICAL (SPMD)
while balancing true causal work; causality differences live in per-core
mask DATA supplied by the host.  K/V for the batch's first 2048 tokens are
recomputed on both cores of a pair (cheaper than any collective here).

Everything runs bf16 with fp32 PSUM accumulation; softmax normalizer via a
ones column appended per head to V (row 64 of the AV PSUM = sum_k P); no
max-subtraction (logits are O(5)).  y^T stays in SBUF; the projection for
the core's own tokens reads it directly.  Output is the core's 1024 token
rows of the final [8192, 1024], reassembled by the host.
"""

import numpy as np
import ml_dtypes

import concourse.bass as bass
import concourse.mybir as mybir
import concourse.tile as tile
from concourse import bacc
from concourse import bass_utils
from concourse.masks import make_identity

F32 = mybir.dt.float32
BF16 = mybir.dt.bfloat16

B, T, C = 4, 2048, 1024
NH, HS = 16, 64
NCORES = 8
NTOK = B * T
P = 128
KO = C // P                 # 8 contraction chunks over C
QW = 512                    # query block width
KC_W = 128                  # kv chunk width (PSUM partition)
GRP = 2                     # kv chunks per exp group
KV_A, KV_B = 1024, 2048     # kv extents of block A / block B
NFC = NH // 2               # 8 feature chunks of 128 (2 heads each)


def build_graph():
    nc = bacc.Bacc(
        "TRN2",
        target_bir_lowering=False,
        debug=False,
        enable_asserts=True,
        num_devices=NCORES,
    )

    xq = nc.dram_tensor("xq", [C, 2 * QW], BF16, kind="ExternalInput").ap()
    xkv = nc.dram_tensor("xkv", [C, KV_B], BF16, kind="ExternalInput").ap()
    w_qkv = nc.dram_tensor("w_qkv", [C, 3 * C], BF16, kind="ExternalInput").ap()
    b_qkv = nc.dram_tensor("b_qkv", [3 * C], F32, kind="ExternalInput").ap()
    w_proj = nc.dram_tensor("w_proj", [C, C], BF16, kind="ExternalInput").ap()
    b_proj = nc.dram_tensor("b_proj", [C], F32, kind="ExternalInput").ap()
    mask_a = nc.dram_tensor("mask_a", [KV_A, QW], BF16, kind="ExternalInput").ap()
    mask_b = nc.dram_tensor("mask_b", [KV_A, QW], BF16, kind="ExternalInput").ap()
    out = nc.dram_tensor("out", [2 * QW, C], F32, kind="ExternalOutput").ap()

    xq_t = xq.rearrange("(ko p) t -> p ko t", p=P)        # [128, 8, 1024]
    xkv_t = xkv.rearrange("(ko p) t -> p ko t", p=P)      # [128, 8, 2048]
    wq_t = w_qkv.rearrange("(ko p) f -> p ko f", p=P)     # [128, 8, 3072]
    wp_t = w_proj.rearrange("(ko p) f -> p ko f", p=P)    # [128, 8, 1024]
    ma_t = mask_a.rearrange("(kc p) q -> p kc q", p=P)    # [128, 8, 512]
    mb_t = mask_b.rearrange("(kc p) q -> p kc q", p=P)    # [128, 8, 512]

    with tile.TileContext(nc) as tc:
        with (
            tc.tile_pool(name="const", bufs=1) as const,
            tc.tile_pool(name="w", bufs=1) as w_pool,
            tc.tile_pool(name="xslab", bufs=2) as xslab_pool,
            tc.tile_pool(name="qk", bufs=1) as qk_pool,
            tc.tile_pool(name="vtok", bufs=1) as v_pool,
            tc.tile_pool(name="pexp", bufs=5) as p_pool,
            tc.tile_pool(name="small", bufs=2) as small_pool,
            tc.tile_pool(name="outsb", bufs=2) as out_pool,
            tc.tile_pool(name="mm_ps", bufs=2, space="PSUM") as mm_ps,
            tc.tile_pool(name="st_ps", bufs=2, space="PSUM") as st_ps,
            tc.tile_pool(name="y_ps", bufs=2, space="PSUM") as y_ps,
        ):
            # ---- QKV pass 1a: q^T for block A only ----
            qT = qk_pool.tile([P, NFC, 2 * QW], BF16, tag="qT")
            w_q = w_pool.tile([P, KO, C], BF16, tag="w", name="w_q")
            for fq in range(NFC):
                nc.sync.dma_start(w_q[:, :, fq * P:(fq + 1) * P],
                                  wq_t[:, :, fq * P:(fq + 1) * P])

            # ---- small constants ----
            bqk_sb = const.tile([P, 24], F32)      # qkv bias, per-partition
            nc.sync.dma_start(bqk_sb[:], b_qkv.rearrange("(c p) -> p c", p=P))
            bp_row = const.tile([1, C], F32)
            nc.sync.dma_start(bp_row[:], b_proj[None, :])
            ones_row = const.tile([1, P], BF16)
            nc.vector.memset(ones_row[:], 1.0)
            ones_col = const.tile([P, KV_B // P, 1], F32)
            nc.vector.memset(ones_col[:], 1.0)

            # v bias broadcast across token partitions: [1,1024] -> [128,1024]
            bv_row = const.tile([1, C], F32)
            nc.sync.dma_start(bv_row[:], b_qkv[None, 2 * C:])
            bv_row16 = const.tile([1, C], BF16)
            nc.vector.tensor_copy(bv_row16[:], bv_row[:])
            bv_bc = const.tile([P, C], F32)
            for half in range(2):
                bv_ps = mm_ps.tile([P, QW], F32, tag="mm")
                nc.tensor.matmul(bv_ps[:], ones_row[:],
                                 bv_row16[:, half * QW:(half + 1) * QW],
                                 start=True, stop=True)
                nc.vector.tensor_copy(bv_bc[:, half * QW:(half + 1) * QW], bv_ps[:])

            # proj bias broadcast across partitions: [1,1024] -> [128,1024]
            bp_row16 = const.tile([1, C], BF16)
            nc.vector.tensor_copy(bp_row16[:], bp_row[:])
            bp_bc = const.tile([P, C], F32)
            for half in range(2):
                bp_ps = mm_ps.tile([P, QW], F32, tag="mm")
                nc.tensor.matmul(bp_ps[:], ones_row[:],
                                 bp_row16[:, half * QW:(half + 1) * QW],
                                 start=True, stop=True)
                nc.vector.tensor_copy(bp_bc[:, half * QW:(half + 1) * QW], bp_ps[:])


            def qt_slab(s, w_tile):
                slab = xslab_pool.tile([P, KO, QW], BF16, tag="xslab",
                                       name=f"xq{s}")
                for kd in range(KO):
                    nc.sync.dma_start(slab[:, kd, :],
                                      xq_t[:, kd, s * QW:(s + 1) * QW])
                for f in range(NFC):
                    ps = mm_ps.tile([P, QW], F32, tag="mm")
                    for k0 in range(KO):
                        nc.tensor.matmul(
                            ps[:], w_tile[:, k0, f * P:(f + 1) * P],
                            slab[:, k0, :],
                            start=(k0 == 0), stop=(k0 == KO - 1),
                        )
                    nc.scalar.activation(
                        qT[:, f, s * QW:(s + 1) * QW], ps[:],
                        mybir.ActivationFunctionType.Identity,
                        bias=bqk_sb[:, f:f + 1],
                    )

            qt_slab(0, w_q)

            # ---- QKV pass 2: k^T and v over 2048 kv tokens ----
            kT = qk_pool.tile([P, NFC, KV_B], BF16, tag="kT")
            v_aug = v_pool.tile([P, KV_B // P, NH * (HS + 1)], BF16, tag="v")
            for h in range(NH):
                nc.vector.tensor_copy(
                    v_aug[:, :, h * (HS + 1) + HS:h * (HS + 1) + HS + 1],
                    ones_col[:])
            w_kv = w_pool.tile([P, KO, 2 * C], BF16, tag="w", name="w_kv")
            for fq in range(2 * NFC):
                nc.sync.dma_start(w_kv[:, :, fq * P:(fq + 1) * P],
                                  wq_t[:, :, C + fq * P:C + (fq + 1) * P])

            def kv_slab(s):
                slab = xslab_pool.tile([P, KO, QW], BF16, tag="xslab",
                                       name=f"xkv{s}")
                for kd in range(KO):
                    nc.sync.dma_start(slab[:, kd, :],
                                      xkv_t[:, kd, s * QW:(s + 1) * QW])
                for f in range(NFC):
                    ps = mm_ps.tile([P, QW], F32, tag="mm")
                    for k0 in range(KO):
                        nc.tensor.matmul(
                            ps[:], w_kv[:, k0, f * P:(f + 1) * P],
                            slab[:, k0, :],
                            start=(k0 == 0), stop=(k0 == KO - 1),
                        )
                    nc.scalar.activation(
                        kT[:, f, s * QW:(s + 1) * QW], ps[:],
                        mybir.ActivationFunctionType.Identity,
                        bias=bqk_sb[:, 8 + f:9 + f],
                    )
                # v token-major directly: lhsT = x slab, rhs = W_v half
                for t4 in range(QW // P):
                    tc_i = s * (QW // P) + t4
                    for nn in range(2):
                        ps = mm_ps.tile([P, QW], F32, tag="mm")
                        for k0 in range(KO):
                            nc.tensor.matmul(
                                ps[:], slab[:, k0, t4 * P:(t4 + 1) * P],
                                w_kv[:, k0, C + nn * QW:C + (nn + 1) * QW],
                                start=(k0 == 0), stop=(k0 == KO - 1),
                            )
                        for hh in range(8):
                            h = 8 * nn + hh
                            nc.vector.tensor_tensor(
                                v_aug[:, tc_i, h * (HS + 1):h * (HS + 1) + HS],
                                ps[:, hh * HS:(hh + 1) * HS],
                                bv_bc[:, h * HS:(h + 1) * HS],
                                mybir.AluOpType.add)

            kv_slab(0)
            kv_slab(1)
            # masks can load behind the first kv slabs
            ma_sb = const.tile([P, KV_A // P, QW], BF16)
            nc.sync.dma_start(ma_sb[:], ma_t)
            mb_sb = const.tile([P, KV_A // P, QW], BF16)
            nc.sync.dma_start(mb_sb[:], mb_t)

            # ---- attention: head pairs packed per feature chunk ----
            # For each fc f, heads e=2f (partitions 0:64) and o=2f+1 (64:128)
            # issue S^T matmuls into slabs j=0/1 of one [128,2,512] PSUM tile
            # with tile_position (0,0)/(64,0) -> concurrent PE row-groups.
            # One exp + one (broadcast) mask op covers both heads.
            yT = qk_pool.tile([P, NFC, 2 * QW], BF16, tag="yT")  # y^T local

            def attn_block(f, blk, nkc, masked_from):
                yps_e = y_ps.tile([P, QW], F32, tag="y", name=f"y_e_{f}_{blk}")
                yps_o = y_ps.tile([P, QW], F32, tag="y", name=f"y_o_{f}_{blk}")
                for kc in range(nkc):
                    stps = st_ps.tile([P, 2, QW], F32, tag="st")
                    for j, hp in ((0, 0), (1, HS)):
                        nc.tensor.matmul(
                            stps[:, j, :],
                            kT[hp:hp + HS, f, kc * KC_W:(kc + 1) * KC_W],
                            qT[hp:hp + HS, f, blk * QW:(blk + 1) * QW],
                            start=True, stop=True,
                            tile_position=(hp, 0),
                        )
                    pexp = p_pool.tile([P, 2, QW], BF16, tag="p")
                    nc.scalar.activation(
                        pexp[:], stps[:],
                        mybir.ActivationFunctionType.Exp,
                        scale=1.0 / np.sqrt(HS),
                    )
                    if blk == 0 or kc >= masked_from:
                        m2 = ma_sb if blk == 0 else mb_sb
                        kcm = kc if blk == 0 else kc - masked_from
                        nc.vector.tensor_tensor(
                            pexp[:], pexp[:],
                            m2[:, kcm:kcm + 1, :].to_broadcast((P, 2, QW)),
                            mybir.AluOpType.mult)
                    for j, h in ((0, 2 * f), (1, 2 * f + 1)):
                        vc = h * (HS + 1)
                        yps = yps_e if j == 0 else yps_o
                        nc.tensor.matmul(
                            yps[0:HS + 1, :],
                            v_aug[:, kc, vc:vc + HS + 1],
                            pexp[:, j, :],
                            start=(kc == 0), stop=(kc == nkc - 1),
                        )
                for j, hp in ((0, 0), (1, HS)):
                    yps = yps_e if j == 0 else yps_o
                    # evict to SBUF right away so the PSUM bank frees for the
                    # next head pair; normalize from SBUF afterwards
                    y_sb = small_pool.tile([HS + 1, QW], F32, tag="y_sb")
                    nc.vector.tensor_copy(y_sb[:], yps[0:HS + 1, :])
                    recip = small_pool.tile([1, QW], BF16, tag="recip")
                    with nc.allow_low_precision(
                            reason="bf16 softmax normalizer within tolerance"):
                        nc.vector.reciprocal(recip[:], y_sb[HS:HS + 1, :])
                    bcps = mm_ps.tile([HS, QW], F32, tag="mm")
                    nc.tensor.matmul(bcps[:], ones_row[:, :HS], recip[:],
                                     start=True, stop=True)
                    bc_sb = small_pool.tile([HS, QW], F32, tag="bc_sb")
                    nc.vector.tensor_copy(bc_sb[:], bcps[:])
                    nc.vector.tensor_tensor(
                        yT[hp:hp + HS, f, blk * QW:(blk + 1) * QW],
                        y_sb[0:HS, :], bc_sb[:], mybir.AluOpType.mult,
                    )

            # block A only needs kv slabs 0-1: interleave its head pairs with
            # the remaining kv production and the block-B q^T pass
            for f in range(NFC // 2):
                attn_block(f, 0, KV_A // KC_W, KV_A // KC_W)
            kv_slab(2)
            for f in range(NFC // 2, NFC):
                attn_block(f, 0, KV_A // KC_W, KV_A // KC_W)
            kv_slab(3)
            w_q2 = w_pool.tile([P, KO, C], BF16, tag="w", name="w_q2")
            for fq in range(NFC):
                nc.sync.dma_start(w_q2[:, :, fq * P:(fq + 1) * P],
                                  wq_t[:, :, fq * P:(fq + 1) * P])
            qt_slab(1, w_q2)
            for f in range(NFC):
                attn_block(f, 1, KV_B // KC_W, KV_A // KC_W)

            # ---- projection for own 1024 tokens (all output columns) ----
            w_p = w_pool.tile([P, KO, C], BF16, tag="w", name="w_p")
            nc.sync.dma_start(w_p[:], wp_t)
            for tm in range(2 * QW // P):      # 8 token chunks of 128
                for nn in range(C // QW):      # 2 column chunks of 512
                    ps = mm_ps.tile([P, QW], F32, tag="mm")
                    for k0 in range(KO):
                        nc.tensor.matmul(
                            ps[:],
                            yT[:, k0, tm * P:(tm + 1) * P],
                            w_p[:, k0, nn * QW:(nn + 1) * QW],
                            start=(k0 == 0), stop=(k0 == KO - 1),
                        )
                    osb = out_pool.tile([P, QW], F32, tag="osb")
                    nc.vector.tensor_tensor(
                        osb[:], ps[:], bp_bc[:, nn * QW:(nn + 1) * QW],
                        mybir.AluOpType.add,
                    )
                    nc.sync.dma_start(
                        out[tm * P:(tm + 1) * P, nn * QW:(nn + 1) * QW], osb[:])

    nc.compile()
    return nc


_NC_CACHE = None


def _get_nc():
    global _NC_CACHE
    if _NC_CACHE is None:
        _NC_CACHE = build_graph()
    return _NC_CACHE


def _q_ranges(c):
    """Global token rows (within [0, 8192)) of core c's blocks A and B."""
    b, z = c // 2, c % 2
    a0 = b * T + 512 * z
    b0 = b * T + T - 512 * (z + 1)
    return (a0, a0 + QW), (b0, b0 + QW)


def make_in_maps(x, W_attn, b_attn, W_proj, b_proj):
    x = np.asarray(x, dtype=np.float32)
    W_attn = np.asarray(W_attn, dtype=np.float32)
    b_attn = np.asarray(b_attn, dtype=np.float32)
    W_proj = np.asarray(W_proj, dtype=np.float32)
    b_proj = np.asarray(b_proj, dtype=np.float32)

    bf = ml_dtypes.bfloat16
    xT = np.ascontiguousarray(x.reshape(NTOK, C).T).astype(bf)  # [1024, 8192]
    wq = np.ascontiguousarray(W_attn).astype(bf)
    wp = np.ascontiguousarray(W_proj).astype(bf)
    kv = np.arange(KV_B)

    in_maps = []
    for c in range(NCORES):
        b, z = c // 2, c % 2
        (a0, a1), (b0, b1) = _q_ranges(c)
        xq_c = np.ascontiguousarray(
            np.concatenate([xT[:, a0:a1], xT[:, b0:b1]], axis=1))
        xkv_c = np.ascontiguousarray(xT[:, b * T:b * T + KV_B])
        qa = np.arange(a0 - b * T, a1 - b * T)   # q positions within batch
        qb = np.arange(b0 - b * T, b1 - b * T)
        m_a = (kv[:KV_A, None] <= qa[None, :]).astype(bf)           # [1024,512]
        m_b = (kv[KV_A:, None] <= qb[None, :]).astype(bf)           # [1024,512]
        in_maps.append({
            "xq": xq_c, "xkv": xkv_c,
            "w_qkv": wq, "b_qkv": b_attn,
            "w_proj": wp, "b_proj": b_proj,
            "mask_a": np.ascontiguousarray(m_a),
            "mask_b": np.ascontiguousarray(m_b),
        })
    return in_maps


def kernel(x, W_attn, b_attn, W_proj, b_proj):
    nc = _get_nc()
    in_maps = make_in_maps(x, W_attn, b_attn, W_proj, b_proj)
    res = bass_utils.run_bass_kernel_spmd(
        nc, in_maps, core_ids=list(range(NCORES)), trace=False,
    )
    out_full = np.empty((NTOK, C), dtype=np.float32)
    for c in range(NCORES):
        (a0, a1), (b0, b1) = _q_ranges(c)
        blk = res.results[c]["out"]
        out_full[a0:a1] = blk[:QW]
        out_full[b0:b1] = blk[QW:]
    kernel.last_results = res
    return out_full.reshape(B, T, C)


# revision 18
# speedup vs baseline: 4518.4835x; 1.0110x over previous
"""Collective-free sequence-sharded causal self-attention for 8 TRN2 cores.

Sharding: core c -> batch b = c//2, zig-zag half z = c%2.  The core computes
ALL 16 heads for two 512-token query blocks of its batch:
    block A: queries [512z, 512z+512)        (kv extent 1024, mask_a)
    block B: queries [2048-512(z+1), ...+512) (kv extent 2048; kv<1024 is
             fully causal -> unmasked; kv in [1024,2048) uses mask_b)
The zig-zag pairing makes every core's instruction graph IDENTICAL (SPMD)
while balancing true causal work; causality differences live in per-core
mask DATA supplied by the host.  K/V for the batch's first 2048 tokens are
recomputed on both cores of a pair (cheaper than any collective here).

Everything runs bf16 with fp32 PSUM accumulation; softmax normalizer via a
ones column appended per head to V (row 64 of the AV PSUM = sum_k P); no
max-subtraction (logits are O(5)).  y^T stays in SBUF; the projection for
the core's own tokens reads it directly.  Output is the core's 1024 token
rows of the final [8192, 1024], reassembled by the host.
"""

import numpy as np
import ml_dtypes

import concourse.bass as bass
import concourse.mybir as mybir
import concourse.tile as tile
from concourse import bacc
from concourse import bass_utils
from concourse.masks import make_identity

F32 = mybir.dt.float32
BF16 = mybir.dt.bfloat16

B, T, C = 4, 2048, 1024
NH, HS = 16, 64
NCORES = 8
NTOK = B * T
P = 128
KO = C // P                 # 8 contraction chunks over C
QW = 512                    # query block width
KC_W = 128                  # kv chunk width (PSUM partition)
GRP = 2                     # kv chunks per exp group
KV_A, KV_B = 1024, 2048     # kv extents of block A / block B
NFC = NH // 2               # 8 feature chunks of 128 (2 heads each)


def build_graph():
    nc = bacc.Bacc(
        "TRN2",
        target_bir_lowering=False,
        debug=False,
        enable_asserts=True,
        num_devices=NCORES,
    )

    xq = nc.dram_tensor("xq", [C, 2 * QW], BF16, kind="ExternalInput").ap()
    xkv = nc.dram_tensor("xkv", [C, KV_B], BF16, kind="ExternalInput").ap()
    w_qkv = nc.dram_tensor("w_qkv", [C, 3 * C], BF16, kind="ExternalInput").ap()
    b_qkv = nc.dram_tensor("b_qkv", [3 * C], F32, kind="ExternalInput").ap()
    w_proj = nc.dram_tensor("w_proj", [C, C], BF16, kind="ExternalInput").ap()
    b_proj = nc.dram_tensor("b_proj", [C], F32, kind="ExternalInput").ap()
    mask_a = nc.dram_tensor("mask_a", [KV_A, QW], BF16, kind="ExternalInput").ap()
    mask_b = nc.dram_tensor("mask_b", [KV_A, QW], BF16, kind="ExternalInput").ap()
    out = nc.dram_tensor("out", [2 * QW, C], F32, kind="ExternalOutput").ap()

    xq_t = xq.rearrange("(ko p) t -> p ko t", p=P)        # [128, 8, 1024]
    xkv_t = xkv.rearrange("(ko p) t -> p ko t", p=P)      # [128, 8, 2048]
    wq_t = w_qkv.rearrange("(ko p) f -> p ko f", p=P)     # [128, 8, 3072]
    wp_t = w_proj.rearrange("(ko p) f -> p ko f", p=P)    # [128, 8, 1024]
    ma_t = mask_a.rearrange("(kc p) q -> p kc q", p=P)    # [128, 8, 512]
    mb_t = mask_b.rearrange("(kc p) q -> p kc q", p=P)    # [128, 8, 512]

    with tile.TileContext(nc) as tc:
        with (
            tc.tile_pool(name="const", bufs=1) as const,
            tc.tile_pool(name="w", bufs=1) as w_pool,
            tc.tile_pool(name="xslab", bufs=2) as xslab_pool,
            tc.tile_pool(name="qk", bufs=1) as qk_pool,
            tc.tile_pool(name="vtok", bufs=1) as v_pool,
            tc.tile_pool(name="pexp", bufs=5) as p_pool,
            tc.tile_pool(name="small", bufs=2) as small_pool,
            tc.tile_pool(name="outsb", bufs=2) as out_pool,
            tc.tile_pool(name="mm_ps", bufs=2, space="PSUM") as mm_ps,
            tc.tile_pool(name="st_ps", bufs=2, space="PSUM") as st_ps,
            tc.tile_pool(name="y_ps", bufs=2, space="PSUM") as y_ps,
        ):
            # ---- QKV pass 1a: q^T for block A only ----
            qT = qk_pool.tile([P, NFC, 2 * QW], BF16, tag="qT")
            w_q = w_pool.tile([P, KO, C], BF16, tag="w", name="w_q")
            for fq in range(NFC):
                nc.sync.dma_start(w_q[:, :, fq * P:(fq + 1) * P],
                                  wq_t[:, :, fq * P:(fq + 1) * P])

            # ---- small constants ----
            bqk_sb = const.tile([P, 24], F32)      # qkv bias, per-partition
            nc.sync.dma_start(bqk_sb[:], b_qkv.rearrange("(c p) -> p c", p=P))
            bp_row = const.tile([1, C], F32)
            nc.sync.dma_start(bp_row[:], b_proj[None, :])
            ones_row = const.tile([1, P], BF16)
            nc.vector.memset(ones_row[:], 1.0)
            ones_col = const.tile([P, KV_B // P, 1], F32)
            nc.vector.memset(ones_col[:], 1.0)

            # v bias broadcast across token partitions: [1,1024] -> [128,1024]
            bv_row = const.tile([1, C], F32)
            nc.sync.dma_start(bv_row[:], b_qkv[None, 2 * C:])
            bv_row16 = const.tile([1, C], BF16)
            nc.vector.tensor_copy(bv_row16[:], bv_row[:])
            bv_bc = const.tile([P, C], F32)
            for half in range(2):
                bv_ps = mm_ps.tile([P, QW], F32, tag="mm")
                nc.tensor.matmul(bv_ps[:], ones_row[:],
                                 bv_row16[:, half * QW:(half + 1) * QW],
                                 start=True, stop=True)
                nc.vector.tensor_copy(bv_bc[:, half * QW:(half + 1) * QW], bv_ps[:])

            # proj bias broadcast across partitions: [1,1024] -> [128,1024]
            bp_row16 = const.tile([1, C], BF16)
            nc.vector.tensor_copy(bp_row16[:], bp_row[:])
            bp_bc = const.tile([P, C], F32)
            for half in range(2):
                bp_ps = mm_ps.tile([P, QW], F32, tag="mm")
                nc.tensor.matmul(bp_ps[:], ones_row[:],
                                 bp_row16[:, half * QW:(half + 1) * QW],
                                 start=True, stop=True)
                nc.vector.tensor_copy(bp_bc[:, half * QW:(half + 1) * QW], bp_ps[:])


            def qt_slab(s, w_tile):
                slab = xslab_pool.tile([P, KO, QW], BF16, tag="xslab",
                                       name=f"xq{s}")
                for kd in range(KO):
                    nc.sync.dma_start(slab[:, kd, :],
                                      xq_t[:, kd, s * QW:(s + 1) * QW])
                for f in range(NFC):
                    ps = mm_ps.tile([P, QW], F32, tag="mm")
                    for k0 in range(KO):
                        nc.tensor.matmul(
                            ps[:], w_tile[:, k0, f * P:(f + 1) * P],
                            slab[:, k0, :],
                            start=(k0 == 0), stop=(k0 == KO - 1),
                        )
                    nc.scalar.activation(
                        qT[:, f, s * QW:(s + 1) * QW], ps[:],
                        mybir.ActivationFunctionType.Identity,
                        bias=bqk_sb[:, f:f + 1],
                    )

            qt_slab(0, w_q)
            qt_slab(1, w_q)

            # ---- QKV pass 2: k^T and v over 2048 kv tokens ----
            kT = qk_pool.tile([P, NFC, KV_B], BF16, tag="kT")
            v_aug = v_pool.tile([P, KV_B // P, NH * (HS + 1)], BF16, tag="v")
            for h in range(NH):
                nc.vector.tensor_copy(
                    v_aug[:, :, h * (HS + 1) + HS:h * (HS + 1) + HS + 1],
                    ones_col[:])
            w_kv = w_pool.tile([P, KO, 2 * C], BF16, tag="w", name="w_kv")
            for fq in range(2 * NFC):
                nc.sync.dma_start(w_kv[:, :, fq * P:(fq + 1) * P],
                                  wq_t[:, :, C + fq * P:C + (fq + 1) * P])

            def kv_slab(s):
                slab = xslab_pool.tile([P, KO, QW], BF16, tag="xslab",
                                       name=f"xkv{s}")
                for kd in range(KO):
                    nc.sync.dma_start(slab[:, kd, :],
                                      xkv_t[:, kd, s * QW:(s + 1) * QW])
                for f in range(NFC):
                    ps = mm_ps.tile([P, QW], F32, tag="mm")
                    for k0 in range(KO):
                        nc.tensor.matmul(
                            ps[:], w_kv[:, k0, f * P:(f + 1) * P],
                            slab[:, k0, :],
                            start=(k0 == 0), stop=(k0 == KO - 1),
                        )
                    nc.scalar.activation(
                        kT[:, f, s * QW:(s + 1) * QW], ps[:],
                        mybir.ActivationFunctionType.Identity,
                        bias=bqk_sb[:, 8 + f:9 + f],
                    )
                # v token-major directly: lhsT = x slab, rhs = W_v half
                for t4 in range(QW // P):
                    tc_i = s * (QW // P) + t4
                    for nn in range(2):
                        ps = mm_ps.tile([P, QW], F32, tag="mm")
                        for k0 in range(KO):
                            nc.tensor.matmul(
                                ps[:], slab[:, k0, t4 * P:(t4 + 1) * P],
                                w_kv[:, k0, C + nn * QW:C + (nn + 1) * QW],
                                start=(k0 == 0), stop=(k0 == KO - 1),
                            )
                        for hh in range(8):
                            h = 8 * nn + hh
                            nc.vector.tensor_tensor(
                                v_aug[:, tc_i, h * (HS + 1):h * (HS + 1) + HS],
                                ps[:, hh * HS:(hh + 1) * HS],
                                bv_bc[:, h * HS:(h + 1) * HS],
                                mybir.AluOpType.add)

            kv_slab(0)
            kv_slab(1)
            # masks can load behind the first kv slabs
            ma_sb = const.tile([P, KV_A // P, QW], BF16)
            nc.sync.dma_start(ma_sb[:], ma_t)
            mb_sb = const.tile([P, KV_A // P, QW], BF16)
            nc.sync.dma_start(mb_sb[:], mb_t)

            # ---- attention: head pairs packed per feature chunk ----
            # For each fc f, heads e=2f (partitions 0:64) and o=2f+1 (64:128)
            # issue S^T matmuls into slabs j=0/1 of one [128,2,512] PSUM tile
            # with tile_position (0,0)/(64,0) -> concurrent PE row-groups.
            # One exp + one (broadcast) mask op covers both heads.
            yT = qk_pool.tile([P, NFC, 2 * QW], BF16, tag="yT")  # y^T local

            def attn_block(f, blk, nkc, masked_from):
                yps_e = y_ps.tile([P, QW], F32, tag="y", name=f"y_e_{f}_{blk}")
                yps_o = y_ps.tile([P, QW], F32, tag="y", name=f"y_o_{f}_{blk}")
                for kc in range(nkc):
                    stps = st_ps.tile([P, 2, QW], F32, tag="st")
                    for j, hp in ((0, 0), (1, HS)):
                        nc.tensor.matmul(
                            stps[:, j, :],
                            kT[hp:hp + HS, f, kc * KC_W:(kc + 1) * KC_W],
                            qT[hp:hp + HS, f, blk * QW:(blk + 1) * QW],
                            start=True, stop=True,
                            tile_position=(hp, 0),
                        )
                    pexp = p_pool.tile([P, 2, QW], BF16, tag="p")
                    nc.scalar.activation(
                        pexp[:], stps[:],
                        mybir.ActivationFunctionType.Exp,
                        scale=1.0 / np.sqrt(HS),
                    )
                    if blk == 0 or kc >= masked_from:
                        m2 = ma_sb if blk == 0 else mb_sb
                        kcm = kc if blk == 0 else kc - masked_from
                        nc.vector.tensor_tensor(
                            pexp[:], pexp[:],
                            m2[:, kcm:kcm + 1, :].to_broadcast((P, 2, QW)),
                            mybir.AluOpType.mult)
                    for j, h in ((0, 2 * f), (1, 2 * f + 1)):
                        vc = h * (HS + 1)
                        yps = yps_e if j == 0 else yps_o
                        nc.tensor.matmul(
                            yps[0:HS + 1, :],
                            v_aug[:, kc, vc:vc + HS + 1],
                            pexp[:, j, :],
                            start=(kc == 0), stop=(kc == nkc - 1),
                        )
                for j, hp in ((0, 0), (1, HS)):
                    yps = yps_e if j == 0 else yps_o
                    # evict to SBUF right away so the PSUM bank frees for the
                    # next head pair; normalize from SBUF afterwards
                    y_sb = small_pool.tile([HS + 1, QW], F32, tag="y_sb")
                    nc.vector.tensor_copy(y_sb[:], yps[0:HS + 1, :])
                    recip = small_pool.tile([1, QW], BF16, tag="recip")
                    with nc.allow_low_precision(
                            reason="bf16 softmax normalizer within tolerance"):
                        nc.vector.reciprocal(recip[:], y_sb[HS:HS + 1, :])
                    bcps = mm_ps.tile([HS, QW], F32, tag="mm")
                    nc.tensor.matmul(bcps[:], ones_row[:, :HS], recip[:],
                                     start=True, stop=True)
                    bc_sb = small_pool.tile([HS, QW], F32, tag="bc_sb")
                    nc.vector.tensor_copy(bc_sb[:], bcps[:])
                    nc.vector.tensor_tensor(
                        yT[hp:hp + HS, f, blk * QW:(blk + 1) * QW],
                        y_sb[0:HS, :], bc_sb[:], mybir.AluOpType.mult,
                    )

            # block A only needs kv slabs 0-1: interleave its head pairs with
            # the remaining kv production and the block-B q^T pass
            for f in range(NFC // 2):
                attn_block(f, 0, KV_A // KC_W, KV_A // KC_W)
            kv_slab(2)
            for f in range(NFC // 2, NFC):
                attn_block(f, 0, KV_A // KC_W, KV_A // KC_W)
            kv_slab(3)
            for f in range(NFC):
                attn_block(f, 1, KV_B // KC_W, KV_A // KC_W)

            # ---- projection for own 1024 tokens (all output columns) ----
            w_p = w_pool.tile([P, KO, C], BF16, tag="w", name="w_p")
            nc.sync.dma_start(w_p[:], wp_t)
            for tm in range(2 * QW // P):      # 8 token chunks of 128
                for nn in range(C // QW):      # 2 column chunks of 512
                    ps = mm_ps.tile([P, QW], F32, tag="mm")
                    for k0 in range(KO):
                        nc.tensor.matmul(
                            ps[:],
                            yT[:, k0, tm * P:(tm + 1) * P],
                            w_p[:, k0, nn * QW:(nn + 1) * QW],
                            start=(k0 == 0), stop=(k0 == KO - 1),
                        )
                    osb = out_pool.tile([P, QW], F32, tag="osb")
                    nc.vector.tensor_tensor(
                        osb[:], ps[:], bp_bc[:, nn * QW:(nn + 1) * QW],
                        mybir.AluOpType.add,
                    )
                    nc.sync.dma_start(
                        out[tm * P:(tm + 1) * P, nn * QW:(nn + 1) * QW], osb[:])

    nc.compile()
    return nc


_NC_CACHE = None


def _get_nc():
    global _NC_CACHE
    if _NC_CACHE is None:
        _NC_CACHE = build_graph()
    return _NC_CACHE


def _q_ranges(c):
    """Global token rows (within [0, 8192)) of core c's blocks A and B."""
    b, z = c // 2, c % 2
    a0 = b * T + 512 * z
    b0 = b * T + T - 512 * (z + 1)
    return (a0, a0 + QW), (b0, b0 + QW)


def make_in_maps(x, W_attn, b_attn, W_proj, b_proj):
    x = np.asarray(x, dtype=np.float32)
    W_attn = np.asarray(W_attn, dtype=np.float32)
    b_attn = np.asarray(b_attn, dtype=np.float32)
    W_proj = np.asarray(W_proj, dtype=np.float32)
    b_proj = np.asarray(b_proj, dtype=np.float32)

    bf = ml_dtypes.bfloat16
    xT = np.ascontiguousarray(x.reshape(NTOK, C).T).astype(bf)  # [1024, 8192]
    wq = np.ascontiguousarray(W_attn).astype(bf)
    wp = np.ascontiguousarray(W_proj).astype(bf)
    kv = np.arange(KV_B)

    in_maps = []
    for c in range(NCORES):
        b, z = c // 2, c % 2
        (a0, a1), (b0, b1) = _q_ranges(c)
        xq_c = np.ascontiguousarray(
            np.concatenate([xT[:, a0:a1], xT[:, b0:b1]], axis=1))
        xkv_c = np.ascontiguousarray(xT[:, b * T:b * T + KV_B])
        qa = np.arange(a0 - b * T, a1 - b * T)   # q positions within batch
        qb = np.arange(b0 - b * T, b1 - b * T)
        m_a = (kv[:KV_A, None] <= qa[None, :]).astype(bf)           # [1024,512]
        m_b = (kv[KV_A:, None] <= qb[None, :]).astype(bf)           # [1024,512]
        in_maps.append({
            "xq": xq_c, "xkv": xkv_c,
            "w_qkv": wq, "b_qkv": b_attn,
            "w_proj": wp, "b_proj": b_proj,
            "mask_a": np.ascontiguousarray(m_a),
            "mask_b": np.ascontiguousarray(m_b),
        })
    return in_maps


def kernel(x, W_attn, b_attn, W_proj, b_proj):
    nc = _get_nc()
    in_maps = make_in_maps(x, W_attn, b_attn, W_proj, b_proj)
    res = bass_utils.run_bass_kernel_spmd(
        nc, in_maps, core_ids=list(range(NCORES)), trace=False,
    )
    out_full = np.empty((NTOK, C), dtype=np.float32)
    for c in range(NCORES):
        (a0, a1), (b0, b1) = _q_ranges(c)
        blk = res.results[c]["out"]
        out_full[a0:a1] = blk[:QW]
        out_full[b0:b1] = blk[QW:]
    kernel.last_results = res
    return out_full.reshape(B, T, C)
